# revision 1
# baseline (speedup 1.0000x reference)
"""Trainium2 Bass kernel for the CGMM E-step (nn_CGMM_89172111000084).

Computes, for stats [U,L,A,C2], transition [L,A,C,C2], layerS [L], arcS [L,A]:
  neighb[u,la]      = where(sum_j stats == 0, 1, sum_j stats)
  rightmost[u,la,c,j] = transition[la,c,j] * stats[u,la,j] / neighb[u,la]
  posterior         = layerS[l]*arcS[l,a] * rightmost
  p_Q[u,c]          = sum_{la,j} posterior[u,la,c,j]

Sharding: U (data parallel) across 8 NeuronCores; params replicated.
Layout: u on SBUF partitions (128/tile), (la,c,j)=2176 on the free dim, so
all big DMAs are fully contiguous. p_Q is computed on the PE via
transpose+matmul to keep the DVE under the DMA roofline.
"""

import sys

for _p in ("/opt/trn_rl_repo", "/root/.axon_site/_ro/trn_rl_repo"):
    if _p not in sys.path:
        sys.path.insert(0, _p)

import numpy as np

U, L, A, C, C2 = 50000, 2, 4, 16, 17
LA = L * A              # 8
K = LA * C2             # 136 free-dim width of stats
N = LA * C * C2         # 2176 free-dim width of rightmost/posterior
NCORES = 8
TILE_P = 128
TILES = 49              # tiles per core
UC = TILE_P * TILES     # 6272 rows per core
UPAD = UC * NCORES      # 50176 (U padded)

_compiled = {}


def _build(reps=1):
    import concourse.bacc as bacc
    import concourse.mybir as mybir
    from concourse.tile import TileContext
    from concourse.masks import make_identity

    F32 = mybir.dt.float32
    AX = mybir.AxisListType
    nc = bacc.Bacc("TRN2")

    stats_d = nc.declare_dram_parameter("stats", [UC, K], F32, isOutput=False)
    t_d = nc.declare_dram_parameter("tfull", [N], F32, isOutput=False)
    w_d = nc.declare_dram_parameter("wfull", [N], F32, isOutput=False)
    g_d = nc.declare_dram_parameter("g", [K, C], F32, isOutput=False)
    rt_d = nc.declare_dram_parameter("rt", [UC, N], F32, isOutput=True)
    post_d = nc.declare_dram_parameter("post", [UC, N], F32, isOutput=True)
    p_d = nc.declare_dram_parameter("p", [UC, C], F32, isOutput=True)

    with TileContext(nc) as tc:
        with (
            tc.tile_pool(name="consts", bufs=1) as cpool,
            tc.tile_pool(name="small", bufs=4) as spool,
            tc.tile_pool(name="big", bufs=3) as bpool,
            tc.tile_pool(name="psum", bufs=2, space="PSUM") as psum,
        ):
            ident = cpool.tile([128, 128], F32)
            make_identity(nc, ident[:])
            tfull = cpool.tile([128, N], F32)
            nc.sync.dma_start(
                out=tfull[:], in_=t_d[:].unsqueeze(0).broadcast_to((128, N))
            )
            wfull = cpool.tile([128, N], F32)
            nc.sync.dma_start(
                out=wfull[:], in_=w_d[:].unsqueeze(0).broadcast_to((128, N))
            )
            ga = cpool.tile([128, C], F32)
            nc.sync.dma_start(out=ga[:], in_=g_d[:128, :])
            gb = cpool.tile([8, C], F32)
            nc.sync.dma_start(out=gb[:], in_=g_d[128:, :])
            pacc = cpool.tile([128, TILES * C], F32)

            t4v = tfull[:].rearrange("p (la c j) -> p la c j", c=C, j=C2)

            for _rep in range(reps):
                for t in range(TILES):
                    st = spool.tile([128, K], F32, tag="stats")
                    nc.gpsimd.dma_start(
                        out=st[:], in_=stats_d[t * 128 : (t + 1) * 128, :]
                    )
                    x3 = st[:].rearrange("p (la j) -> p la j", j=C2)

                    nb = spool.tile([128, LA], F32, tag="nb")
                    nc.vector.reduce_sum(nb[:], x3, axis=AX.X)
                    mask = spool.tile([128, LA], F32, tag="mask")
                    nc.vector.tensor_scalar(
                        out=mask[:], in0=nb[:], scalar1=0.0, scalar2=None,
                        op0=mybir.AluOpType.is_equal,
                    )
                    nc.vector.tensor_add(out=nb[:], in0=nb[:], in1=mask[:])
                    rcp = spool.tile([128, LA], F32, tag="rcp")
                    nc.vector.reciprocal(rcp[:], nb[:])

                    ss = spool.tile([128, K], F32, tag="ss")
                    ss3 = ss[:].rearrange("p (la j) -> p la j", j=C2)
                    nc.vector.tensor_mul(
                        out=ss3, in0=x3,
                        in1=rcp[:, :, None].broadcast_to((128, LA, C2)),
                    )

                    rt = bpool.tile([128, N], F32, tag="rt")
                    rt4 = rt[:].rearrange("p (la c j) -> p la c j", c=C, j=C2)
                    nc.vector.tensor_mul(
                        out=rt4, in0=t4v,
                        in1=ss3[:, :, None, :].broadcast_to((128, LA, C, C2)),
                    )
                    post = bpool.tile([128, N], F32, tag="post")
                    nc.vector.tensor_mul(out=post[:], in0=rt[:], in1=wfull[:])

                    # p_Q on PE: p[u,c] = sum_k g[k,c] * ss[u,k]
                    ssta = psum.tile([128, 128], F32, tag="ssta")
                    nc.tensor.transpose(ssta[:], ss[:, :128], ident[:])
                    sstb = psum.tile([8, 128], F32, tag="sstb")
                    nc.tensor.transpose(sstb[:], ss[:, 128:], ident[:])
                    ssta_sb = spool.tile([128, 128], F32, tag="ssta_sb")
                    nc.scalar.copy(ssta_sb[:], ssta[:])
                    sstb_sb = spool.tile([8, 128], F32, tag="sstb_sb")
                    nc.scalar.copy(sstb_sb[:], sstb[:])
                    pps = psum.tile([16, 128], F32, tag="pps")
                    nc.tensor.matmul(
                        pps[:], lhsT=ga[:], rhs=ssta_sb[:], start=True, stop=False
                    )
                    nc.tensor.matmul(
                        pps[:], lhsT=gb[:], rhs=sstb_sb[:], start=False, stop=True
                    )
                    p16 = spool.tile([16, 128], F32, tag="p16")
                    nc.scalar.copy(p16[:], pps[:])
                    ppt = psum.tile([128, 16], F32, tag="ppt")
                    nc.tensor.transpose(ppt[:], p16[:], ident[:16, :16])
                    nc.scalar.copy(pacc[:, t * C : (t + 1) * C], ppt[:])

                    nc.sync.dma_start(
                        out=rt_d[t * 128 : (t + 1) * 128, :], in_=rt[:]
                    )
                    nc.scalar.dma_start(
                        out=post_d[t * 128 : (t + 1) * 128, :], in_=post[:]
                    )

                nc.sync.dma_start(
                    out=p_d[:].rearrange("(t p) c -> p t c", p=128),
                    in_=pacc[:].rearrange("p (t c) -> p t c", c=C),
                )

    nc.compile()
    return nc


def _get_compiled(reps=1):
    if reps not in _compiled:
        _compiled[reps] = _build(reps)
    return _compiled[reps]


def _prep_inputs(stats, layerS, arcS, transition):
    stats = np.asarray(stats, dtype=np.float32).reshape(U, K)
    w = (
        np.asarray(layerS, np.float32)[:, None] * np.asarray(arcS, np.float32)
    ).reshape(LA)
    tr = np.asarray(transition, np.float32).reshape(LA, C, C2)
    tfull = np.ascontiguousarray(tr.reshape(N))
    wfull = np.repeat(w, C * C2)
    g = np.ascontiguousarray((w[:, None, None] * tr).transpose(0, 2, 1).reshape(K, C))
    stats_pad = np.zeros((UPAD, K), np.float32)
    stats_pad[:U] = stats
    return [
        {
            "stats": np.ascontiguousarray(stats_pad[i * UC : (i + 1) * UC]),
            "tfull": tfull,
            "wfull": wfull,
            "g": g,
        }
        for i in range(NCORES)
    ]


def kernel(stats, layerS, arcS, transition):
    from concourse.bass_utils import run_bass_kernel_spmd

    in_maps = _prep_inputs(stats, layerS, arcS, transition)
    nc = _get_compiled()
    res = run_bass_kernel_spmd(nc, in_maps, core_ids=list(range(NCORES))).results
    rt = np.concatenate([r["rt"] for r in res], 0)[:U].reshape(U, L, A, C, C2)
    post = np.concatenate([r["post"] for r in res], 0)[:U].reshape(U, L, A, C, C2)
    p = np.concatenate([r["p"] for r in res], 0)[:U]
    return (p, post, rt)
